# revision 18
# baseline (speedup 1.0000x reference)
"""EqualizedModConv2D (StyleGAN2 modulated conv) on 8 TRN2 NeuronCores.

Winograd F(2x2, 3x3) formulation (exact algebra; bf16 only on matmul
operands):
    mod[n,i]  = style[n] @ (fc_weight * fc_scale).T[.,i] + bias[i] + 1
    out[n]    = dem[n,:] * conv2d(mod[n,:] * x[n], w_scale * weight)
    dem[n,o]  = 1 / sqrt( sum_i mod[n,i]^2 * wsq[o,i] + eps )
    wsq[o,i]  = w_scale^2 * sum_{kh,kw} weight[o,i,kh,kw]^2   (host precomp)

The conv itself runs in the Winograd domain: with U = G w G^T precomputed
on the host (pure function of the weights, shipped as bf16), the device
computes per 4x4 input tile d (stride 2):  y = A^T [ U .* (B^T d B) ] A.
B^T/A^T entries are 0/+-1, so the input/output transforms are adds of
strided slices (DVE), and the 16 frequency-domain matmuls contract over
in-channels on the PE:  Y[f][o,t] = sum_i U[f][i,o] V[f][i,t].  That is
256 PE matmuls of 512 rows instead of the 576 a direct 3x3 conv needs
(2.25x fewer MACs).

Sharding: data-parallel over batch N=16 -> 2 samples per core; weights
replicated.

Per-core pipeline:
  ACT: modulate+pad x (transposed layout), demod-scale PSUM->bf16 copies
  DVE: input transform (2 passes of 4 adds), output transform (A^T folds)
  Pool: zeroing the padded-x tiles
  PE : 16 mod/4 demod matmuls + 64 Winograd matmuls per oc-block
"""

import numpy as np

import concourse.bass as bass
import concourse.bacc as bacc
import concourse.tile as tile
from concourse import mybir
from concourse.bass_utils import run_bass_kernel_spmd

F32 = mybir.dt.float32
BF16 = mybir.dt.bfloat16
AF = mybir.ActivationFunctionType

N_FULL, IC, OC, H, W = 16, 512, 512, 32, 32
DLAT, KS = 512, 3
NCORES = 8
NPC = N_FULL // NCORES          # samples per core
HP, WP = H + 2, W + 2           # padded image
FC_SCALE = 1.0 / float(np.sqrt(DLAT))
W_SCALE = 1.0 / float(np.sqrt(IC * KS * KS))
EPS = 1e-8
NIB = IC // 128
NOB = OC // 128
NDB = DLAT // 128
NT = 16                         # winograd tiles per image dim (32/2)

_NC = None
VARIANT = "full"   # full | notrans | noout | convonly  (timing experiments)


def _build(loop_iters=None):
    nc = bacc.Bacc()
    x_d = nc.declare_dram_parameter("x", [NPC, IC, H, W], F32, False)
    # u: per oc-block: [i_local, f(=4*fy+fx), icb, o_local], bf16
    u_d = nc.declare_dram_parameter("u", [NOB, 128, 16, NIB, 128], BF16, False)
    # pk packs [fcwT (512c) | styleT (NPC c) | bias (1c)] along the free dim
    pk_d = nc.declare_dram_parameter("pk", [DLAT, IC + NPC + 1], F32, False)
    # wq: wsqT[i, o] = w_scale^2 * sum_k weight[o,i,k]^2
    wq_d = nc.declare_dram_parameter("wq", [NIB, 128, OC], F32, False)
    out_d = nc.declare_dram_parameter("out", [NPC, OC, H, W], F32, True)

    import contextlib
    with tile.TileContext(nc) as tc:
        with (tc.For_i(0, loop_iters, 1,
                       hint_engines=(mybir.EngineType.PE,
                                     mybir.EngineType.Activation,
                                     mybir.EngineType.DVE,
                                     mybir.EngineType.Pool,
                                     mybir.EngineType.SP))
              if loop_iters else contextlib.nullcontext()):
         with (
            tc.tile_pool(name="const", bufs=1) as cpool,
            tc.tile_pool(name="xr", bufs=3) as xr_pool,
            tc.tile_pool(name="xpt", bufs=3) as xpt_pool,
            tc.tile_pool(name="pst", bufs=2) as p_pool,
            tc.tile_pool(name="ut", bufs=2) as u_pool,
            tc.tile_pool(name="yb", bufs=18) as yb_pool,
            tc.tile_pool(name="qt", bufs=2) as q_pool,
            tc.tile_pool(name="tmp", bufs=4) as tmp_pool,
            tc.tile_pool(name="obf", bufs=2) as obf_pool,
            tc.tile_pool(name="small", bufs=4) as small_pool,
            tc.tile_pool(name="cpsum", bufs=7, space="PSUM") as cpsum_pool,
            tc.tile_pool(name="spsum", bufs=1, space="PSUM") as spsum_pool,
        ):
            # ---------------- constants ----------------
            fcw_sb, st_sb, b1_sb = [], [], []
            for d in range(NDB):
                ps = cpool.tile([128, IC + NPC + 1], F32, tag=f"pk{d}",
                                name=f"pk{d}")
                nc.sync.dma_start(out=ps[:], in_=pk_d[d * 128:(d + 1) * 128, :])
                fcw_sb.append(ps)
                st_sb.append(ps[:, IC:IC + NPC])
                t1 = cpool.tile([128, 1], F32, tag=f"b1{d}", name=f"b1{d}")
                nc.vector.tensor_scalar_add(t1[:], ps[:, IC + NPC:IC + NPC + 1], 1.0)
                b1_sb.append(t1)

            wq_sb = []
            for i in range(NIB):
                wq = cpool.tile([128, OC], F32, tag=f"wq{i}", name=f"wq{i}")
                nc.sync.dma_start(out=wq[:], in_=wq_d[i])
                wq_sb.append(wq)

            eps_sb = cpool.tile([128, 1], F32, tag="eps", name="eps")
            nc.vector.memset(eps_sb[:], float(EPS))

            # prefetch first U block
            u_sb = []
            for o in range(NOB):
                ut = u_pool.tile([128, 16, NIB, 128], BF16, tag="ut",
                                 name=f"ut{o}")
                u_sb.append(ut)
            nc.sync.dma_start(out=u_sb[0][:], in_=u_d[0])

            # ---------------- mod / mod^2  (i on partitions, n free) --------
            mod_sb, mod2_sb = [], []
            for i in range(NIB):
                mp = spsum_pool.tile([128, NPC], F32, tag="sp", name=f"mp{i}")
                for d in range(NDB):
                    nc.tensor.matmul(
                        mp[:],
                        fcw_sb[d][:, i * 128:(i + 1) * 128],
                        st_sb[d],
                        start=(d == 0),
                        stop=(d == NDB - 1),
                    )
                m = cpool.tile([128, NPC], F32, tag=f"mod{i}", name=f"mod{i}")
                nc.scalar.activation(m[:], mp[:], AF.Identity,
                                     bias=b1_sb[i][:, 0:1], scale=FC_SCALE)
                m2 = cpool.tile([128, NPC], F32, tag=f"mod2{i}", name=f"mod2{i}")
                nc.scalar.square(m2[:], m[:])
                mod_sb.append(m)
                mod2_sb.append(m2)

            # ---------------- input transform -> V[icb][i, f, n, ty, tx] ---
            v_sb = []
            for i in range(NIB):
                v = cpool.tile([128, 16, NPC, NT, NT], BF16, tag=f"v{i}",
                               name=f"v{i}")
                v_sb.append(v)

            if VARIANT in ("notrans", "convonly"):
                for i in range(NIB):
                    nc.vector.memset(v_sb[i][:], 0.25)
            do_trans = VARIANT in ("full", "noout", "trans_only",
                                   "smallxdma",
                                   "trans_nodve", "trans_s1",
                                   "trans_dvememset", "trans_noset")
            for i in range(NIB) if do_trans else []:
                for n in range(NPC):
                    xr = xr_pool.tile([128, H, W], F32, tag="xr",
                                      name=f"xr{i}_{n}")
                    if VARIANT == "smallxdma":
                        nc.gpsimd.dma_start(
                            out=xr[:, 0:1, :],
                            in_=x_d[n, i * 128:(i + 1) * 128, 0:1, :])
                    else:
                        # Pool-queue DGE: keeps x loads off the SP queue so
                        # next-iteration input DMAs aren't stuck behind this
                        # iteration's output DMAs.
                        nc.gpsimd.dma_start(out=xr[:],
                                            in_=x_d[n, i * 128:(i + 1) * 128, :, :])
                    # padded, modulated image with y split by parity:
                    # xpt[i, w(34), par(2), yy(17)];  y = 2*yy + par
                    # ACT eats the transposed/strided writes; every DVE AP
                    # below then has a contiguous inner dim.
                    xpt = xpt_pool.tile([128, WP, 2, 17], BF16, tag="xpt",
                                        name=f"xpt{i}_{n}")
                    if VARIANT == "trans_dvememset":
                        nc.vector.memset(xpt[:], 0.0)
                    elif VARIANT != "trans_noset":
                        # only the pad borders need zeroing; the interior is
                        # fully overwritten by the two modulate writes
                        nc.gpsimd.memset(xpt[:, 0:WP:WP - 1], 0.0)
                        nc.gpsimd.memset(xpt[:, 1:W + 1, 0, 0:1], 0.0)
                        nc.gpsimd.memset(xpt[:, 1:W + 1, 1, 16:17], 0.0)
                    # even padded rows y=2yy (yy 1..16) <- image rows 1,3..31
                    nc.scalar.mul(
                        xpt[:, 1:W + 1, 0, 1:17].transpose([0, 2, 1]),
                        xr[:, 1:32:2, :], mod_sb[i][:, n:n + 1])
                    # odd padded rows y=2yy+1 (yy 0..15) <- image rows 0,2..30
                    nc.scalar.mul(
                        xpt[:, 1:W + 1, 1, 0:16].transpose([0, 2, 1]),
                        xr[:, 0:32:2, :], mod_sb[i][:, n:n + 1])
                    if VARIANT == "trans_nodve":
                        continue
                    # stage 1 (along w): P[i, fx, tx, par, yy]
                    P = p_pool.tile([128, 4, NT, 2, 17], BF16, tag="p",
                                    name=f"p{i}_{n}")
                    ev0 = xpt[:, 0:32:2]
                    od1 = xpt[:, 1:33:2]
                    ev2 = xpt[:, 2:34:2]
                    od3 = xpt[:, 3:34:2]
                    nc.vector.tensor_sub(P[:, 0], ev0, ev2)
                    nc.vector.tensor_add(P[:, 1], od1, ev2)
                    nc.vector.tensor_sub(P[:, 2], ev2, od1)
                    nc.vector.tensor_sub(P[:, 3], od1, od3)
                    if VARIANT == "trans_s1":
                        continue
                    # stage 2 (along y): V[i, 4*fy+fx, n, tx, ty]
                    r0 = P[:, :, :, 0, 0:16]   # y = 0,2..30
                    r1 = P[:, :, :, 1, 0:16]   # y = 1,3..31
                    r2 = P[:, :, :, 0, 1:17]   # y = 2,4..32
                    r3 = P[:, :, :, 1, 1:17]   # y = 3,5..33
                    for fy, (a, b, op) in enumerate(
                            [(r0, r2, "sub"), (r1, r2, "add"),
                             (r2, r1, "sub"), (r1, r3, "sub")]):
                        oa = v_sb[i][:, 4 * fy:4 * fy + 4, n, :, :]
                        if op == "add":
                            nc.vector.tensor_add(oa, a, b)
                        else:
                            nc.vector.tensor_sub(oa, a, b)

            # ---------------- per-oc-block: demod + winograd conv ----------
            t_only = VARIANT.startswith("trans_")
            if t_only:
                for o in range(NOB):
                    obf = obf_pool.tile([128, NPC, H, W], F32, tag="ob",
                                        name=f"ob{o}")
                    nc.vector.memset(obf[:], 0.0)
                    for n in range(NPC):
                        nc.sync.dma_start(
                            out=out_d[n, o * 128:(o + 1) * 128, :, :],
                            in_=obf[:, n])
            for o in range(NOB) if not t_only else []:
                if o + 1 < NOB:
                    nc.sync.dma_start(out=u_sb[o + 1][:], in_=u_d[o + 1])
                ut = u_sb[o]

                dp = spsum_pool.tile([128, NPC], F32, tag="sp", name=f"dp{o}")
                for i in range(NIB):
                    nc.tensor.matmul(dp[:],
                                     wq_sb[i][:, o * 128:(o + 1) * 128],
                                     mod2_sb[i][:],
                                     start=(i == 0), stop=(i == NIB - 1))
                sq = small_pool.tile([128, NPC], F32, tag="sq", name=f"sq{o}")
                nc.scalar.activation(sq[:], dp[:], AF.Sqrt,
                                     bias=eps_sb[:, 0:1], scale=1.0)
                dem = small_pool.tile([128, NPC], F32, tag="dem",
                                      name=f"dem{o}")
                nc.vector.reciprocal(dem[:], sq[:])

                do_out = VARIANT in ("full", "notrans")
                Q = q_pool.tile([128, 2, 4, NPC * NT * NT], BF16, tag="q",
                                name=f"q{o}")
                for fx in range(4):
                    yb4 = []
                    for fy in range(4):
                        f = 4 * fy + fx
                        yp = cpsum_pool.tile([128, NPC, NT * NT], F32,
                                             tag="cps", name=f"cps{o}_{f}")
                        for i in range(NIB):
                            nc.tensor.matmul(yp[:], ut[:, f, i, :],
                                             v_sb[i][:, f],
                                             start=(i == 0),
                                             stop=(i == NIB - 1))
                        yb = yb_pool.tile([128, NPC, NT * NT], BF16, tag="yb",
                                          name=f"yb{o}_{f}")
                        for n in range(NPC):
                            nc.scalar.mul(yb[:, n], yp[:, n],
                                          dem[:, n:n + 1])
                        yb4.append(yb[:].rearrange("p n t -> p (n t)"))
                    if not do_out:
                        continue
                    # Q[dy, fx] = A^T fold over fy (on bf16 copies)
                    t0 = tmp_pool.tile([128, NPC * NT * NT], BF16, tag="tm",
                                       name=f"t0_{o}_{fx}")
                    nc.vector.tensor_add(t0[:], yb4[0], yb4[1])
                    nc.vector.tensor_add(Q[:, 0, fx], t0[:], yb4[2])
                    t1 = tmp_pool.tile([128, NPC * NT * NT], BF16, tag="tm",
                                       name=f"t1_{o}_{fx}")
                    nc.vector.tensor_sub(t1[:], yb4[1], yb4[2])
                    nc.vector.tensor_sub(Q[:, 1, fx], t1[:], yb4[3])

                # output stage 2: y[dy,dx] = A^T fold over fx.  DVE writes
                # contiguous yd tiles; ACT scatters into the (h, w) layout.
                obf = obf_pool.tile([128, NPC, H, W], F32, tag="ob",
                                    name=f"ob{o}")
                obr = obf[:].rearrange("p n (t a) (u b) -> p n t a u b",
                                       a=2, b=2)
                if not do_out:
                    nc.vector.memset(obf[:], 0.0)
                for dy in range(2) if do_out else []:
                    q = [Q[:, dy, k] for k in range(4)]
                    for dx in range(2):
                        t2 = tmp_pool.tile([128, NPC * NT * NT], BF16,
                                           tag="tm", name=f"t2_{o}_{dy}{dx}")
                        yd = tmp_pool.tile([128, NPC * NT * NT], BF16,
                                           tag="yd", name=f"yd_{o}_{dy}{dx}")
                        if dx == 0:
                            nc.vector.tensor_add(t2[:], q[0], q[1])
                            nc.vector.tensor_add(yd[:], t2[:], q[2])
                        else:
                            nc.vector.tensor_sub(t2[:], q[1], q[2])
                            nc.vector.tensor_sub(yd[:], t2[:], q[3])
                        # col index of yd is (n, tx, ty); pixel = (2ty+dy,
                        # 2tx+dx)
                        nc.scalar.copy(
                            obr[:, :, :, dy, :, dx],
                            yd[:].rearrange("p (n u t) -> p n u t",
                                            n=NPC, u=NT).transpose([0, 1, 3, 2]))

                for n in range(NPC):
                    nc.sync.dma_start(
                        out=out_d[n, o * 128:(o + 1) * 128, :, :],
                        in_=obf[:, n],
                    )
    nc.finalize()
    return nc


def _get_nc():
    global _NC
    if _NC is None:
        _NC = _build()
    return _NC


def _winograd_weights(weight):
    G = np.array([[1, 0, 0], [.5, .5, .5], [.5, -.5, .5], [0, 0, 1]],
                 np.float64)
    wk = (W_SCALE * np.asarray(weight, np.float64))       # (OC, IC, 3, 3)
    U = np.einsum('ac,oicd,ed->aeoi', G, wk, G)           # (4,4,OC,IC)
    U6 = U.reshape(4, 4, NOB, 128, NIB, 128)              # fy,fx,ocb,ol,icb,il
    up = U6.transpose(2, 5, 0, 1, 4, 3).reshape(NOB, 128, 16, NIB, 128)
    import jax.numpy as jnp
    return np.ascontiguousarray(
        np.asarray(jnp.asarray(up.astype(np.float32)).astype(jnp.bfloat16)))


def _make_in_maps(x, style, weight, fc_weight, bias):
    x = np.ascontiguousarray(np.asarray(x, np.float32))
    u = _winograd_weights(weight)
    w64 = np.asarray(weight, np.float64)
    wsqT = (W_SCALE * W_SCALE) * (w64 * w64).sum(axis=(2, 3)).T  # (IC, OC)
    wq = np.ascontiguousarray(
        wsqT.astype(np.float32).reshape(NIB, 128, OC))
    styleT = np.asarray(style, np.float32).T
    fcwT = np.asarray(fc_weight, np.float32).T
    biasr = np.asarray(bias, np.float32).reshape(IC, 1)
    in_maps = []
    for c in range(NCORES):
        pk = np.ascontiguousarray(np.concatenate(
            [fcwT, styleT[:, c * NPC:(c + 1) * NPC], biasr], axis=1))
        in_maps.append({
            "x": np.ascontiguousarray(x[c * NPC:(c + 1) * NPC]),
            "u": u,
            "pk": pk,
            "wq": wq,
        })
    return in_maps


def _run(in_maps, trace=False):
    last = None
    for _ in range(3):
        try:
            return run_bass_kernel_spmd(_get_nc(), in_maps, list(range(NCORES)),
                                        trace=trace)
        except Exception as e:  # transient NRT/device errors: retry
            last = e
    raise last


def kernel(x, style, weight, fc_weight, bias):
    br = _run(_make_in_maps(x, style, weight, fc_weight, bias))
    out = np.concatenate([br.results[c]["out"] for c in range(NCORES)], axis=0)
    return out


def _make_runner(nc, in_maps):
    import jax
    import numpy as np
    from jax.sharding import Mesh, PartitionSpec
    from jax.experimental.shard_map import shard_map
    from concourse import mybir as _mb
    from concourse.bass2jax import (_bass_exec_p, install_neuronx_cc_hook,
                                    partition_id_tensor)
    install_neuronx_cc_hook()
    n_cores = len(in_maps)
    partition_name = nc.partition_id_tensor.name if nc.partition_id_tensor else None
    in_names, out_names, out_avals, zero_outs = [], [], [], []
    for alloc in nc.m.functions[0].allocations:
        if not isinstance(alloc, _mb.MemoryLocationSet):
            continue
        name = alloc.memorylocations[0].name
        if alloc.kind == "ExternalInput":
            if name != partition_name:
                in_names.append(name)
        elif alloc.kind == "ExternalOutput":
            shape = tuple(alloc.tensor_shape)
            dtype = _mb.dt.np(alloc.dtype)
            out_avals.append(jax.core.ShapedArray(shape, dtype))
            out_names.append(name)
            zero_outs.append(np.zeros(shape, dtype))
    n_params = len(in_names)
    all_in_names = list(in_names) + list(out_names)
    if partition_name is not None:
        all_in_names.append(partition_name)

    def _body(*args):
        operands = list(args)
        if partition_name is not None:
            operands.append(partition_id_tensor())
        outs = _bass_exec_p.bind(
            *operands,
            out_avals=tuple(out_avals),
            in_names=tuple(all_in_names),
            out_names=tuple(out_names),
            lowering_input_output_aliases=(),
            sim_require_finite=True,
            sim_require_nnan=True,
            nc=nc,
        )
        return tuple(outs)

    devices = jax.devices()[:n_cores]
    mesh = Mesh(np.asarray(devices), ("core",))
    in_specs = (PartitionSpec("core"),) * (n_params + len(out_names))
    out_specs = (PartitionSpec("core"),) * len(out_names)
    fn = jax.jit(shard_map(_body, mesh=mesh, in_specs=in_specs,
                           out_specs=out_specs, check_rep=False))
    concat = []
    for nm in in_names:
        per = [np.asarray(in_maps[c][nm]) for c in range(n_cores)]
        concat.append(np.concatenate(per, axis=0))
    concat += [np.zeros((n_cores * z.shape[0], *z.shape[1:]), z.dtype)
               for z in zero_outs]
    args = [jax.device_put(a) for a in concat]
    return fn, args


def _time_runner(fn, args, iters, reps):
    import time
    import jax
    o = fn(*args)
    jax.block_until_ready(o)  # compile + warm
    best = float("inf")
    for _ in range(reps):
        t0 = time.perf_counter()
        for _ in range(iters):
            o = fn(*args)
            jax.block_until_ready(o)
        best = min(best, (time.perf_counter() - t0) / iters)
    return best


_NC_LOOP = None
_LOOP_R = 128


def measure_hw(inputs, iters=6, reps=3):
    """Differential HW timing: wall(body x R in a hardware loop) minus
    wall(body x 1), divided by R-1. Removes the ~120 ms axon dispatch
    overhead. Returns (per_iter_ns, (wall_plain_ns, wall_loop_ns))."""
    global _NC_LOOP
    in_maps = _make_in_maps(**inputs)
    fn1, args1 = _make_runner(_get_nc(), in_maps)
    if _NC_LOOP is None:
        _NC_LOOP = _build(loop_iters=_LOOP_R)
    fnR, argsR = _make_runner(_NC_LOOP, in_maps)
    w1 = _time_runner(fn1, args1, iters, reps) * 1e9
    wR = _time_runner(fnR, argsR, iters, reps) * 1e9
    per_iter = (wR - w1) / (_LOOP_R - 1)
    return per_iter, (w1, wR)


def predict_ns():
    """Cost-model (TimelineSim) predicted single-core kernel duration in ns."""
    from concourse.timeline_sim import TimelineSim
    ts = TimelineSim(_get_nc(), no_exec=True)
    return ts.simulate()


def run_profiled(inputs):
    """Dev helper: run with NTFF tracing; returns BassKernelResults."""
    return _run(_make_in_maps(**inputs), trace=True)
